# revision 16
# baseline (speedup 1.0000x reference)
"""Trainium2 Bass kernel for nn_Alpha2Assoc: 3-layer alpha compositing,
hybrid ACT/DVE architecture, all-bf16.

Two sub-kernels over disjoint pixel sets, interleaved so engine streams
overlap:

A-path (rows 0..R_A): partition-major [(b,d)=128, pix]. Per layer:
  u = Ln(1-a) on ScalarE, exclusive cumsum over d via TensorE matmul
  with a block-triangular 0/1 stationary, vis = Exp(PSUM) on ScalarE.
  occ/a/out muls on VectorE (bf16 2x/4x modes). ScalarE-bound:
  6 passes at ~1 elem/cyc.

B-path (rows R_A..64): pixel-major [pix=128, d-segments of 32 along free
  dim]. Exclusive cumprod computed DIRECTLY by VectorE
  tensor_tensor_scan: state = max(ta[t-1]*state, bnd[t]) with
  bnd = 1 at segment starts (exact reset since all products <= 1),
  ta read through a 1-shifted view of a leading-zero-column buffer.
  Each scan is split into two chained chunks (initial = prev chunk's
  last column) so a scan never blocks the DVE queue for ~5us at a
  stretch. All B work on VectorE.

Scheduling: a dummy 8-wide Ln is issued first so the ~2.7us ACT table
load overlaps the first input DMA; tri/bnd const DMAs queue after the
first A-tile's input DMA; B-path groups are paced over 6 slots/round,
delayed past the ramp (DELAY_SLOTS) so early scans don't block the
A-path's occ/a muls that feed Ln2/Ln3.

Rejected via HW A/B: scalar_tensor_tensor fusion (1x-only on DVE, made
DVE slower than TS4x+TT2x), GPSIMD offload (elementwise ~2.5x slower
than modeled AND contends for the DVE SBUF port, +28% on scans),
PS_N=1024/4-buf PSUM (ACT per-instruction overhead outweighed the
stall savings).

The split ratio R_B/64 balances ScalarE (A) against VectorE (A muls +
all of B). bf16 I/O halves DMA traffic; scaled-max err ~1.27e-2 vs
2e-2 budget.
"""

import numpy as np
import ml_dtypes

import concourse.bass as bass
import concourse.tile as tile
from concourse import bacc, mybir
from concourse._compat import with_exitstack
from concourse.bass_utils import run_bass_kernel_spmd

BF16 = ml_dtypes.bfloat16

# --- Pin Ln/Exp to the one table set containing both -------------------
_orig_get_activation_tables = bacc.get_activation_tables


def _pinned_get_activation_tables(arch):
    tables = _orig_get_activation_tables(arch)
    keep = {mybir.ActivationFunctionType.Ln, mybir.ActivationFunctionType.Exp}
    return {
        name: (fns if name == "natural_log_exp_and_others" else fns - keep)
        for name, fns in tables.items()
    }


bacc.get_activation_tables = _pinned_get_activation_tables

# --- Enable walrus LDWEIGHTS dedup (identical tri stationary) ----------
import concourse.bass_utils as _bu

# (ldw-opt dedup disabled: walrus visitInstLdweights asserts on this
# kernel's bf16 stationary; PE has headroom without it)

B, D, H, W = 4, 32, 512, 512
N_CORES = 8
H_SH = H // N_CORES              # 64 rows per core
P = B * D                        # 128 partitions
SEG = D                          # cumprod segment length in B layout

R_B = 14                         # rows on the B (scan) path, of 64
R_A = H_SH - R_B
N_A = R_A * W
N_B = R_B * W

TILE_A = 4096
_lead = [2048, 2048]
_rest = N_A - sum(_lead)
A_SIZES = _lead + [TILE_A] * (_rest // TILE_A) + ([_rest % TILE_A] if _rest % TILE_A else [])
A_OFFS = [sum(A_SIZES[:k]) for k in range(len(A_SIZES))]
NB_TILES = 3
_bt = (N_B // NB_TILES) // SEG * SEG
B_SIZES = [_bt] * (NB_TILES - 1) + [N_B - _bt * (NB_TILES - 1)]
B_OFFS = [sum(B_SIZES[:k]) for k in range(NB_TILES)]
B_TILE = max(B_SIZES)
MM_CHUNK = 512
PS_N = 2048

F32 = mybir.dt.float32
BF = mybir.dt.bfloat16
AF = mybir.ActivationFunctionType
OP = mybir.AluOpType

_COMPILED = {}


def _tri_matrix() -> np.ndarray:
    k = np.arange(P)
    m = np.arange(P)
    same_b = (k[:, None] // D) == (m[None, :] // D)
    lower = (k[:, None] % D) < (m[None, :] % D)
    return (same_b & lower).astype(np.float32)


@with_exitstack
def _alpha_kernel(ctx, tc, outA_aps, outB_aps, inA_ap, inB_ap, tri_ap, bnd_ap):
    nc = tc.nc
    const_pool = ctx.enter_context(tc.tile_pool(name="const", bufs=1))
    # A-path pools (bf16, TILE_A wide)
    a_pool = ctx.enter_context(tc.tile_pool(name="a", bufs=2))
    u_pool = ctx.enter_context(tc.tile_pool(name="u", bufs=2))
    vis_pool = ctx.enter_context(tc.tile_pool(name="vis", bufs=5))
    s_pool = ctx.enter_context(tc.tile_pool(name="s", bufs=3))
    occ_pool = ctx.enter_context(tc.tile_pool(name="occ", bufs=3))
    o_pool = ctx.enter_context(tc.tile_pool(name="o", bufs=3))
    psum_pool = ctx.enter_context(tc.tile_pool(name="ps", bufs=2, space="PSUM"))
    # B-path pools
    ab_pool = ctx.enter_context(tc.tile_pool(name="ab", bufs=2))
    ta_pool = ctx.enter_context(tc.tile_pool(name="ta", bufs=2))
    visb_pool = ctx.enter_context(tc.tile_pool(name="visb", bufs=2))
    nvb_pool = ctx.enter_context(tc.tile_pool(name="nvb", bufs=2))
    sb_pool = ctx.enter_context(tc.tile_pool(name="sb", bufs=2))
    ob_pool = ctx.enter_context(tc.tile_pool(name="ob", bufs=2))

    # tiny dummy activation: forces the Ln/Exp ACT table load (~2.7us) to
    # overlap the first input DMA instead of stalling the first real Ln
    warm = const_pool.tile([P, 8], BF)
    nc.vector.memset(warm[:], 0.0)
    nc.scalar.activation(warm[:], warm[:], AF.Ln, bias=1.0, scale=-1.0)

    tri = const_pool.tile([P, P], BF)
    bnd = const_pool.tile([P, max(B_SIZES)], BF)

    def load_consts():
        nc.sync.dma_start(tri[:], tri_ap[:, :])
        nc.sync.dma_start(bnd[:], bnd_ap[:, 0:max(B_SIZES)])

    # pre-zero the leading column of every ta buffer (shifted-view reset
    # reads it; writes only ever touch cols 1..N so it stays zero)
    for _ in range(2):
        t = ta_pool.tile([P, B_TILE + 1], BF, tag="ta")
        nc.vector.memset(t[:, 0:1], 0.0)

    def cumsum_mm(u, n):
        """Issue matmuls for one layer-tile; return psum tiles for exp."""
        pss = []
        for h in range((n + PS_N - 1) // PS_N):
            w = min(PS_N, n - h * PS_N)
            ps = psum_pool.tile([P, PS_N], F32, tag="ps")
            for j in range((w + MM_CHUNK - 1) // MM_CHUNK):
                mc = min(MM_CHUNK, w - j * MM_CHUNK)
                nc.tensor.matmul(
                    ps[:, bass.ds(j * MM_CHUNK, mc)],
                    tri[:],
                    u[:, bass.ds(h * PS_N + j * MM_CHUNK, mc)],
                    start=True,
                    stop=True,
                )
            pss.append((ps, w))
        return pss

    def exp_drain(pss, vis):
        off = 0
        for ps, w in pss:
            nc.scalar.activation(
                vis[:, bass.ds(off, w)], ps[:, bass.ds(0, w)], AF.Exp
            )
            off += w

    # ---------------- A-path stages (software-pipelined over tiles) ----
    # Ln/matmul issue is separated from the Exp drain so the ACT stream
    # always has a ready Ln between a layer's matmuls and its Exps
    # (otherwise ACT idles ~1us per layer-tile waiting on TensorE).
    st = {}

    def st_a_ln(i, after_dma=None):
        n = A_SIZES[i]
        sl = bass.ds(A_OFFS[i], n)
        a1 = a_pool.tile([P, n], BF, tag="a")
        nsp = 4 if n >= 2048 else 2
        hh = n // nsp
        for k in range(nsp):
            w = hh if k < nsp - 1 else n - hh * (nsp - 1)
            nc.sync.dma_start(a1[:, bass.ds(k * hh, w)],
                              inA_ap[:, bass.ds(A_OFFS[i] + k * hh, w)])
        if after_dma is not None:
            after_dma()
        u1 = u_pool.tile([P, n], BF, tag="u")
        nc.scalar.activation(u1[:], a1[:], AF.Ln, bias=1.0, scale=-1.0)
        st[i] = {"a1": a1, "ps1": cumsum_mm(u1, n)}

    def st_a_exp(i):
        n = A_SIZES[i]
        sl = bass.ds(A_OFFS[i], n)
        vis1 = vis_pool.tile([P, n], BF, tag="vis")
        exp_drain(st[i].pop("ps1"), vis1)
        nc.sync.dma_start(outA_aps[0][:, sl], vis1[:])
        st[i]["vis1"] = vis1

    def st_b_pre(i):
        n = A_SIZES[i]
        a1, vis1 = st[i]["a1"], st[i]["vis1"]
        occ1 = occ_pool.tile([P, n], BF, tag="occ")
        nc.vector.tensor_scalar(occ1[:], vis1[:], -1.0, 1.0, OP.mult, OP.add)
        a2 = s_pool.tile([P, n], BF, tag="s")
        nc.vector.tensor_mul(a2[:], a1[:], occ1[:])
        u2 = u_pool.tile([P, n], BF, tag="u")
        nc.scalar.activation(u2[:], a2[:], AF.Ln, bias=1.0, scale=-1.0)
        st[i].update({"a2": a2, "occ1": occ1, "ps2": cumsum_mm(u2, n)})

    def st_b_exp(i):
        n = A_SIZES[i]
        sl = bass.ds(A_OFFS[i], n)
        vis2 = vis_pool.tile([P, n], BF, tag="vis")
        exp_drain(st[i].pop("ps2"), vis2)
        o2 = o_pool.tile([P, n], BF, tag="o")
        nc.vector.tensor_mul(o2[:], vis2[:], st[i]["occ1"][:])
        nc.sync.dma_start(outA_aps[1][:, sl], o2[:])
        st[i]["vis2"] = vis2

    def st_c_pre(i):
        n = A_SIZES[i]
        a2, vis2 = st[i]["a2"], st[i]["vis2"]
        occ2 = occ_pool.tile([P, n], BF, tag="occ")
        nc.vector.tensor_scalar(occ2[:], vis2[:], -1.0, 1.0, OP.mult, OP.add)
        a3 = s_pool.tile([P, n], BF, tag="s")
        nc.vector.tensor_mul(a3[:], a2[:], occ2[:])
        u3 = u_pool.tile([P, n], BF, tag="u")
        nc.scalar.activation(u3[:], a3[:], AF.Ln, bias=1.0, scale=-1.0)
        st[i].update({"occ2": occ2, "ps3": cumsum_mm(u3, n)})

    def st_c_exp(i):
        n = A_SIZES[i]
        sl = bass.ds(A_OFFS[i], n)
        vis3 = vis_pool.tile([P, n], BF, tag="vis")
        exp_drain(st[i].pop("ps3"), vis3)
        o3 = o_pool.tile([P, n], BF, tag="o")
        nc.vector.tensor_mul(o3[:], vis3[:], st[i]["occ2"][:])
        nc.sync.dma_start(outA_aps[2][:, sl], o3[:])
        del st[i]

    # ---------------- B-path op groups (generator of closures) --------
    def b_groups():
        for j in range(len(B_SIZES)):
            n = B_SIZES[j]
            sl = bass.ds(B_OFFS[j], n)
            stb = {}

            def g1(j=j, n=n, sl=sl, stb=stb):
                a1b = ab_pool.tile([P, n], BF, tag="ab")
                h = n // 2
                nc.sync.dma_start(a1b[:, 0:h], inB_ap[:, bass.ds(B_OFFS[j], h)])
                nc.sync.dma_start(a1b[:, h:n], inB_ap[:, bass.ds(B_OFFS[j] + h, n - h)])
                ta = ta_pool.tile([P, B_TILE + 1], BF, tag="ta")
                nc.vector.tensor_scalar(ta[:, 1:n + 1], a1b[:], -1.0, 1.0,
                                        OP.mult, OP.add)
                stb.update(a1b=a1b, ta=ta)

            def g2a(j=j, n=n, sl=sl, stb=stb):
                h = (n // 2) // SEG * SEG
                vis1 = visb_pool.tile([P, n], BF, tag="visb")
                nc.vector.tensor_tensor_scan(vis1[:, 0:h], stb["ta"][:, 0:h],
                                             bnd[:, 0:h], 1.0, OP.mult, OP.max)
                stb["vis1"] = vis1

            def g2(j=j, n=n, sl=sl, stb=stb):
                h = (n // 2) // SEG * SEG
                vis1 = stb["vis1"]
                nc.vector.tensor_tensor_scan(vis1[:, h:n], stb["ta"][:, h:n],
                                             bnd[:, h:n], vis1[:, h - 1:h],
                                             OP.mult, OP.max)
                nc.sync.dma_start(outB_aps[0][:, sl], vis1[:])

            def g3(j=j, n=n, sl=sl, stb=stb):
                nv1 = nvb_pool.tile([P, n], BF, tag="nvb")
                nc.vector.tensor_scalar(nv1[:], stb["vis1"][:], -1.0, 1.0,
                                        OP.mult, OP.add)
                a2 = sb_pool.tile([P, n], BF, tag="sb")
                nc.vector.tensor_mul(a2[:], stb["a1b"][:], nv1[:])
                ta = ta_pool.tile([P, B_TILE + 1], BF, tag="ta")
                nc.vector.tensor_scalar(ta[:, 1:n + 1], a2[:], -1.0, 1.0,
                                        OP.mult, OP.add)
                stb.update(nv1=nv1, a2=a2, ta=ta)

            def g4a(j=j, n=n, sl=sl, stb=stb):
                h = (n // 2) // SEG * SEG
                vis2 = visb_pool.tile([P, n], BF, tag="visb")
                nc.vector.tensor_tensor_scan(vis2[:, 0:h], stb["ta"][:, 0:h],
                                             bnd[:, 0:h], 1.0, OP.mult, OP.max)
                stb["vis2"] = vis2

            def g4(j=j, n=n, sl=sl, stb=stb):
                h = (n // 2) // SEG * SEG
                vis2 = stb["vis2"]
                nc.vector.tensor_tensor_scan(vis2[:, h:n], stb["ta"][:, h:n],
                                             bnd[:, h:n], vis2[:, h - 1:h],
                                             OP.mult, OP.max)

            def g5(j=j, n=n, sl=sl, stb=stb):
                o2 = ob_pool.tile([P, n], BF, tag="ob")
                nc.vector.tensor_mul(o2[:], stb["vis2"][:], stb["nv1"][:])
                nc.sync.dma_start(outB_aps[1][:, sl], o2[:])
                nv2 = nvb_pool.tile([P, n], BF, tag="nvb")
                nc.vector.tensor_scalar(nv2[:], stb["vis2"][:], -1.0, 1.0,
                                        OP.mult, OP.add)
                stb["nv2"] = nv2

            def g6(j=j, n=n, sl=sl, stb=stb):
                a3 = sb_pool.tile([P, n], BF, tag="sb")
                nc.vector.tensor_mul(a3[:], stb["a2"][:], stb["nv2"][:])
                ta = ta_pool.tile([P, B_TILE + 1], BF, tag="ta")
                nc.vector.tensor_scalar(ta[:, 1:n + 1], a3[:], -1.0, 1.0,
                                        OP.mult, OP.add)
                stb["ta"] = ta

            def g7a(j=j, n=n, sl=sl, stb=stb):
                h = (n // 2) // SEG * SEG
                vis3 = visb_pool.tile([P, n], BF, tag="visb")
                nc.vector.tensor_tensor_scan(vis3[:, 0:h], stb["ta"][:, 0:h],
                                             bnd[:, 0:h], 1.0, OP.mult, OP.max)
                stb["vis3"] = vis3

            def g7(j=j, n=n, sl=sl, stb=stb):
                h = (n // 2) // SEG * SEG
                vis3 = stb["vis3"]
                nc.vector.tensor_tensor_scan(vis3[:, h:n], stb["ta"][:, h:n],
                                             bnd[:, h:n], vis3[:, h - 1:h],
                                             OP.mult, OP.max)

            def g8(j=j, n=n, sl=sl, stb=stb):
                o3 = ob_pool.tile([P, n], BF, tag="ob")
                nc.vector.tensor_mul(o3[:], stb["vis3"][:], stb["nv2"][:])
                nc.sync.dma_start(outB_aps[2][:, sl], o3[:])

            yield from (g1, g2a, g2, g3, g4a, g4, g5, g6, g7a, g7, g8)

    bq = b_groups()
    n_groups = 11 * len(B_SIZES)
    n_slots = 6 * (len(A_SIZES) + 2)
    popped = [0, 0]  # groups popped, slots seen

    DELAY_SLOTS = 8
    eff_slots = n_slots - DELAY_SLOTS

    def pop_b():
        popped[1] += 1
        s = popped[1] - DELAY_SLOTS
        want = 0 if s <= 0 else (s * n_groups + eff_slots - 1) // eff_slots
        while popped[0] < want:
            try:
                next(bq)()
            except StopIteration:
                return
            popped[0] += 1

    # ---------------- schedule -----------------------------------------
    NT = len(A_SIZES)
    for r in range(NT + 2):
        if r < NT:
            st_a_ln(r, after_dma=load_consts if r == 0 else None)
        pop_b()
        if 1 <= r <= NT:
            st_b_pre(r - 1)
        pop_b()
        if r < NT:
            st_a_exp(r)
        pop_b()
        if 2 <= r:
            st_c_pre(r - 2)
        pop_b()
        if 1 <= r <= NT:
            st_b_exp(r - 1)
        pop_b()
        if 2 <= r:
            st_c_exp(r - 2)
        pop_b()
    for _ in range(11 * len(B_SIZES)):
        pop_b()


def _build():
    nc = bacc.Bacc("TRN2", target_bir_lowering=False, debug=False,
                   num_devices=N_CORES)
    inA_ap = nc.dram_tensor("alphaA", [P, N_A], BF, kind="ExternalInput").ap()
    inB_ap = nc.dram_tensor("alphaB", [P, N_B], BF, kind="ExternalInput").ap()
    tri_ap = nc.dram_tensor("tri", [P, P], BF, kind="ExternalInput").ap()
    bnd_ap = nc.dram_tensor("bnd", [P, max(B_SIZES)], BF,
                            kind="ExternalInput").ap()
    outA_aps = [
        nc.dram_tensor(f"outA{c}", [P, N_A], BF, kind="ExternalOutput").ap()
        for c in range(3)
    ]
    outB_aps = [
        nc.dram_tensor(f"outB{c}", [P, N_B], BF, kind="ExternalOutput").ap()
        for c in range(3)
    ]
    with tile.TileContext(nc) as tc:
        _alpha_kernel(tc, outA_aps, outB_aps, inA_ap, inB_ap, tri_ap, bnd_ap)
    nc.compile()
    return nc


def _get_nc():
    if "nc" not in _COMPILED:
        _COMPILED["nc"] = _build()
    return _COMPILED["nc"]


def _run(alpha_imgs: np.ndarray, trace: bool = False):
    nc = _get_nc()
    tri = _tri_matrix().astype(BF16)
    bndv = np.zeros((P, max(B_SIZES)), dtype=np.float32)
    bndv[:, 0::SEG] = 1.0
    bndv = bndv.astype(BF16)
    # clamp below 1: bf16 rounding can hit 1.0 exactly, and the A-path's
    # Ln(1-a) -> -inf would poison the matmul (0 * -inf = NaN)
    a = np.minimum(np.asarray(alpha_imgs)[:, :, 0], np.float32(1 - 2**-8))
    a = a.astype(BF16)  # [B, D, H, W] bf16
    in_maps = []
    for c in range(N_CORES):
        h0 = c * H_SH
        shA = np.ascontiguousarray(a[:, :, h0:h0 + R_A, :]).reshape(P, N_A)
        # B rows -> pixel-major: [B, D, R_B, W] -> [B, R_B, W, D] -> [128, -1]
        shB = np.ascontiguousarray(
            a[:, :, h0 + R_A:h0 + H_SH, :].transpose(0, 2, 3, 1)
        ).reshape(P, N_B)
        in_maps.append({"alphaA": shA, "alphaB": shB, "tri": tri, "bnd": bndv})
    res = None
    backoffs = [3.0, 10.0, 30.0, 60.0]
    for attempt in range(len(backoffs) + 1):
        try:
            res = run_bass_kernel_spmd(
                nc, in_maps, core_ids=list(range(N_CORES)), trace=trace
            )
            break
        except Exception:
            if attempt == len(backoffs):
                raise
            import time

            time.sleep(backoffs[attempt])
    out = np.empty((B, D, 3, H, W), dtype=np.float32)
    for c in range(N_CORES):
        r = res.results[c]
        h0 = c * H_SH
        oA = np.stack([r["outA0"], r["outA1"], r["outA2"]], axis=1)
        oA = oA.astype(np.float32).reshape(B, D, 3, R_A, W)
        out[:, :, :, h0:h0 + R_A, :] = oA
        # [128, 3, N_B] pixel-major -> [B, D, 3, R_B, W]
        oB = np.stack([r["outB0"], r["outB1"], r["outB2"]], axis=1)
        oB = oB.astype(np.float32)
        ob = oB.reshape(P, 3, N_B // SEG, SEG)           # [part, l, px/part, d]
        ob = ob.transpose(1, 0, 2, 3).reshape(3, B, R_B, W, SEG)
        out[:, :, :, h0 + R_A:h0 + H_SH, :] = ob.transpose(1, 4, 0, 2, 3)
    return out, res


def kernel(alpha_imgs: np.ndarray) -> np.ndarray:
    out, _ = _run(alpha_imgs, trace=False)
    return out



# revision 17
# speedup vs baseline: 1.0060x; 1.0060x over previous
"""Trainium2 Bass kernel for nn_Alpha2Assoc: 3-layer alpha compositing,
hybrid ACT/DVE architecture, all-bf16.

Two sub-kernels over disjoint pixel sets, interleaved so engine streams
overlap:

A-path (rows 0..R_A): partition-major [(b,d)=128, pix]. Per layer:
  u = Ln(1-a) on ScalarE, exclusive cumsum over d via TensorE matmul
  with a block-triangular 0/1 stationary, vis = Exp(PSUM) on ScalarE.
  occ/a/out muls on VectorE (bf16 2x/4x modes). ScalarE-bound:
  6 passes at ~1 elem/cyc.

B-path (rows R_A..64): pixel-major [pix=128, d-segments of 32 along free
  dim]. Exclusive cumprod computed DIRECTLY by VectorE
  tensor_tensor_scan: state = max(ta[t-1]*state, bnd[t]) with
  bnd = 1 at segment starts (exact reset since all products <= 1),
  ta read through a 1-shifted view of a leading-zero-column buffer.
  Each scan is split into two chained chunks (initial = prev chunk's
  last column) so a scan never blocks the DVE queue for ~5us at a
  stretch. All B work on VectorE.

Scheduling: a dummy 8-wide Ln is issued first so the ~2.7us ACT table
load overlaps the first input DMA; tri/bnd const DMAs queue after the
first A-tile's input DMA; B-path groups are paced over 6 slots/round,
delayed past the ramp (DELAY_SLOTS) so early scans don't block the
A-path's occ/a muls that feed Ln2/Ln3.

Rejected via HW A/B: scalar_tensor_tensor fusion (1x-only on DVE, made
DVE slower than TS4x+TT2x), GPSIMD offload (elementwise ~2.5x slower
than modeled AND contends for the DVE SBUF port, +28% on scans),
PS_N=1024/4-buf PSUM (ACT per-instruction overhead outweighed the
stall savings).

The split ratio R_B/64 balances ScalarE (A) against VectorE (A muls +
all of B). bf16 I/O halves DMA traffic; scaled-max err ~1.27e-2 vs
2e-2 budget.
"""

import numpy as np
import ml_dtypes

import concourse.bass as bass
import concourse.tile as tile
from concourse import bacc, mybir
from concourse._compat import with_exitstack
from concourse.bass_utils import run_bass_kernel_spmd

BF16 = ml_dtypes.bfloat16

# --- Pin Ln/Exp to the one table set containing both -------------------
_orig_get_activation_tables = bacc.get_activation_tables


def _pinned_get_activation_tables(arch):
    tables = _orig_get_activation_tables(arch)
    keep = {mybir.ActivationFunctionType.Ln, mybir.ActivationFunctionType.Exp}
    return {
        name: (fns if name == "natural_log_exp_and_others" else fns - keep)
        for name, fns in tables.items()
    }


bacc.get_activation_tables = _pinned_get_activation_tables

# --- Enable walrus LDWEIGHTS dedup (identical tri stationary) ----------
import concourse.bass_utils as _bu

# (ldw-opt dedup disabled: walrus visitInstLdweights asserts on this
# kernel's bf16 stationary; PE has headroom without it)

B, D, H, W = 4, 32, 512, 512
N_CORES = 8
H_SH = H // N_CORES              # 64 rows per core
P = B * D                        # 128 partitions
SEG = D                          # cumprod segment length in B layout

R_B = 14                         # rows on the B (scan) path, of 64
R_A = H_SH - R_B
N_A = R_A * W
N_B = R_B * W

TILE_A = 4096
_lead = [2048, 2048]
_rest = N_A - sum(_lead)
A_SIZES = _lead + [TILE_A] * (_rest // TILE_A) + ([_rest % TILE_A] if _rest % TILE_A else [])
A_OFFS = [sum(A_SIZES[:k]) for k in range(len(A_SIZES))]
NB_TILES = 3
_bt = (N_B // NB_TILES) // SEG * SEG
B_SIZES = [_bt] * (NB_TILES - 1) + [N_B - _bt * (NB_TILES - 1)]
B_OFFS = [sum(B_SIZES[:k]) for k in range(NB_TILES)]
B_TILE = max(B_SIZES)
MM_CHUNK = 512
PS_N = 2048

F32 = mybir.dt.float32
BF = mybir.dt.bfloat16
AF = mybir.ActivationFunctionType
OP = mybir.AluOpType

_COMPILED = {}


def _tri_matrix() -> np.ndarray:
    k = np.arange(P)
    m = np.arange(P)
    same_b = (k[:, None] // D) == (m[None, :] // D)
    lower = (k[:, None] % D) < (m[None, :] % D)
    return (same_b & lower).astype(np.float32)


@with_exitstack
def _alpha_kernel(ctx, tc, outA_aps, outB_aps, inA_ap, inB_ap, tri_ap, bnd_ap):
    nc = tc.nc
    const_pool = ctx.enter_context(tc.tile_pool(name="const", bufs=1))
    # A-path pools (bf16, TILE_A wide)
    a_pool = ctx.enter_context(tc.tile_pool(name="a", bufs=2))
    u_pool = ctx.enter_context(tc.tile_pool(name="u", bufs=2))
    vis_pool = ctx.enter_context(tc.tile_pool(name="vis", bufs=5))
    s_pool = ctx.enter_context(tc.tile_pool(name="s", bufs=3))
    occ_pool = ctx.enter_context(tc.tile_pool(name="occ", bufs=3))
    o_pool = ctx.enter_context(tc.tile_pool(name="o", bufs=3))
    psum_pool = ctx.enter_context(tc.tile_pool(name="ps", bufs=2, space="PSUM"))
    # B-path pools
    ab_pool = ctx.enter_context(tc.tile_pool(name="ab", bufs=2))
    ta_pool = ctx.enter_context(tc.tile_pool(name="ta", bufs=2))
    visb_pool = ctx.enter_context(tc.tile_pool(name="visb", bufs=2))
    nvb_pool = ctx.enter_context(tc.tile_pool(name="nvb", bufs=2))
    sb_pool = ctx.enter_context(tc.tile_pool(name="sb", bufs=2))
    ob_pool = ctx.enter_context(tc.tile_pool(name="ob", bufs=2))

    # tiny dummy activation: forces the Ln/Exp ACT table load (~2.7us) to
    # overlap the first input DMA instead of stalling the first real Ln
    warm = const_pool.tile([P, 8], BF)
    nc.vector.memset(warm[:], 0.0)
    nc.scalar.activation(warm[:], warm[:], AF.Ln, bias=1.0, scale=-1.0)

    tri = const_pool.tile([P, P], BF)
    bnd = const_pool.tile([P, max(B_SIZES)], BF)

    def load_consts():
        nc.sync.dma_start(tri[:], tri_ap[:, :])
        nc.sync.dma_start(bnd[:], bnd_ap[:, 0:max(B_SIZES)])

    # pre-zero the leading column of every ta buffer (shifted-view reset
    # reads it; writes only ever touch cols 1..N so it stays zero)
    for _ in range(2):
        t = ta_pool.tile([P, B_TILE + 1], BF, tag="ta")
        nc.vector.memset(t[:, 0:1], 0.0)

    def cumsum_mm(u, n):
        """Issue matmuls for one layer-tile; return psum tiles for exp."""
        pss = []
        for h in range((n + PS_N - 1) // PS_N):
            w = min(PS_N, n - h * PS_N)
            ps = psum_pool.tile([P, PS_N], F32, tag="ps")
            for j in range((w + MM_CHUNK - 1) // MM_CHUNK):
                mc = min(MM_CHUNK, w - j * MM_CHUNK)
                nc.tensor.matmul(
                    ps[:, bass.ds(j * MM_CHUNK, mc)],
                    tri[:],
                    u[:, bass.ds(h * PS_N + j * MM_CHUNK, mc)],
                    start=True,
                    stop=True,
                )
            pss.append((ps, w))
        return pss

    def exp_drain(pss, vis):
        off = 0
        for ps, w in pss:
            nc.scalar.activation(
                vis[:, bass.ds(off, w)], ps[:, bass.ds(0, w)], AF.Exp
            )
            off += w

    # ---------------- A-path stages (software-pipelined over tiles) ----
    # Ln/matmul issue is separated from the Exp drain so the ACT stream
    # always has a ready Ln between a layer's matmuls and its Exps
    # (otherwise ACT idles ~1us per layer-tile waiting on TensorE).
    st = {}

    def st_a_ln(i, after_dma=None):
        n = A_SIZES[i]
        sl = bass.ds(A_OFFS[i], n)
        a1 = a_pool.tile([P, n], BF, tag="a")
        nsp = 4 if n >= 2048 else 2
        hh = n // nsp
        for k in range(nsp):
            w = hh if k < nsp - 1 else n - hh * (nsp - 1)
            nc.sync.dma_start(a1[:, bass.ds(k * hh, w)],
                              inA_ap[:, bass.ds(A_OFFS[i] + k * hh, w)])
        if after_dma is not None:
            after_dma()
        u1 = u_pool.tile([P, n], BF, tag="u")
        nc.scalar.activation(u1[:], a1[:], AF.Ln, bias=1.0, scale=-1.0)
        st[i] = {"a1": a1, "ps1": cumsum_mm(u1, n)}

    def st_a_exp(i):
        n = A_SIZES[i]
        vis1 = vis_pool.tile([P, n], BF, tag="vis")
        off = 0
        for ps, w in st[i].pop("ps1"):
            nc.scalar.activation(vis1[:, bass.ds(off, w)],
                                 ps[:, bass.ds(0, w)], AF.Exp)
            nc.sync.dma_start(outA_aps[0][:, bass.ds(A_OFFS[i] + off, w)],
                              vis1[:, bass.ds(off, w)])
            off += w
        st[i]["vis1"] = vis1

    def st_b_pre(i):
        n = A_SIZES[i]
        a1, vis1 = st[i]["a1"], st[i]["vis1"]
        occ1 = occ_pool.tile([P, n], BF, tag="occ")
        nc.vector.tensor_scalar(occ1[:], vis1[:], -1.0, 1.0, OP.mult, OP.add)
        a2 = s_pool.tile([P, n], BF, tag="s")
        nc.vector.tensor_mul(a2[:], a1[:], occ1[:])
        u2 = u_pool.tile([P, n], BF, tag="u")
        nc.scalar.activation(u2[:], a2[:], AF.Ln, bias=1.0, scale=-1.0)
        st[i].update({"a2": a2, "occ1": occ1, "ps2": cumsum_mm(u2, n)})

    def st_b_exp(i):
        n = A_SIZES[i]
        vis2 = vis_pool.tile([P, n], BF, tag="vis")
        o2 = o_pool.tile([P, n], BF, tag="o")
        occ1 = st[i]["occ1"]
        off = 0
        for ps, w in st[i].pop("ps2"):
            c = bass.ds(off, w)
            nc.scalar.activation(vis2[:, c], ps[:, bass.ds(0, w)], AF.Exp)
            nc.vector.tensor_mul(o2[:, c], vis2[:, c], occ1[:, c])
            nc.sync.dma_start(outA_aps[1][:, bass.ds(A_OFFS[i] + off, w)],
                              o2[:, c])
            off += w
        st[i]["vis2"] = vis2

    def st_c_pre(i):
        n = A_SIZES[i]
        a2, vis2 = st[i]["a2"], st[i]["vis2"]
        occ2 = occ_pool.tile([P, n], BF, tag="occ")
        nc.vector.tensor_scalar(occ2[:], vis2[:], -1.0, 1.0, OP.mult, OP.add)
        a3 = s_pool.tile([P, n], BF, tag="s")
        nc.vector.tensor_mul(a3[:], a2[:], occ2[:])
        u3 = u_pool.tile([P, n], BF, tag="u")
        nc.scalar.activation(u3[:], a3[:], AF.Ln, bias=1.0, scale=-1.0)
        st[i].update({"occ2": occ2, "ps3": cumsum_mm(u3, n)})

    def st_c_exp(i):
        n = A_SIZES[i]
        vis3 = vis_pool.tile([P, n], BF, tag="vis")
        o3 = o_pool.tile([P, n], BF, tag="o")
        occ2 = st[i]["occ2"]
        off = 0
        for ps, w in st[i].pop("ps3"):
            c = bass.ds(off, w)
            nc.scalar.activation(vis3[:, c], ps[:, bass.ds(0, w)], AF.Exp)
            nc.vector.tensor_mul(o3[:, c], vis3[:, c], occ2[:, c])
            nc.sync.dma_start(outA_aps[2][:, bass.ds(A_OFFS[i] + off, w)],
                              o3[:, c])
            off += w
        del st[i]

    # ---------------- B-path op groups (generator of closures) --------
    def b_groups():
        for j in range(len(B_SIZES)):
            n = B_SIZES[j]
            sl = bass.ds(B_OFFS[j], n)
            stb = {}

            def g1(j=j, n=n, sl=sl, stb=stb):
                a1b = ab_pool.tile([P, n], BF, tag="ab")
                h = n // 2
                nc.sync.dma_start(a1b[:, 0:h], inB_ap[:, bass.ds(B_OFFS[j], h)])
                nc.sync.dma_start(a1b[:, h:n], inB_ap[:, bass.ds(B_OFFS[j] + h, n - h)])
                ta = ta_pool.tile([P, B_TILE + 1], BF, tag="ta")
                nc.vector.tensor_scalar(ta[:, 1:n + 1], a1b[:], -1.0, 1.0,
                                        OP.mult, OP.add)
                stb.update(a1b=a1b, ta=ta)

            def g2a(j=j, n=n, sl=sl, stb=stb):
                h = (n // 2) // SEG * SEG
                vis1 = visb_pool.tile([P, n], BF, tag="visb")
                nc.vector.tensor_tensor_scan(vis1[:, 0:h], stb["ta"][:, 0:h],
                                             bnd[:, 0:h], 1.0, OP.mult, OP.max)
                stb["vis1"] = vis1

            def g2(j=j, n=n, sl=sl, stb=stb):
                h = (n // 2) // SEG * SEG
                vis1 = stb["vis1"]
                nc.vector.tensor_tensor_scan(vis1[:, h:n], stb["ta"][:, h:n],
                                             bnd[:, h:n], vis1[:, h - 1:h],
                                             OP.mult, OP.max)
                nc.sync.dma_start(outB_aps[0][:, sl], vis1[:])

            def g3(j=j, n=n, sl=sl, stb=stb):
                nv1 = nvb_pool.tile([P, n], BF, tag="nvb")
                nc.vector.tensor_scalar(nv1[:], stb["vis1"][:], -1.0, 1.0,
                                        OP.mult, OP.add)
                a2 = sb_pool.tile([P, n], BF, tag="sb")
                nc.vector.tensor_mul(a2[:], stb["a1b"][:], nv1[:])
                ta = ta_pool.tile([P, B_TILE + 1], BF, tag="ta")
                nc.vector.tensor_scalar(ta[:, 1:n + 1], a2[:], -1.0, 1.0,
                                        OP.mult, OP.add)
                stb.update(nv1=nv1, a2=a2, ta=ta)

            def g4a(j=j, n=n, sl=sl, stb=stb):
                h = (n // 2) // SEG * SEG
                vis2 = visb_pool.tile([P, n], BF, tag="visb")
                nc.vector.tensor_tensor_scan(vis2[:, 0:h], stb["ta"][:, 0:h],
                                             bnd[:, 0:h], 1.0, OP.mult, OP.max)
                stb["vis2"] = vis2

            def g4(j=j, n=n, sl=sl, stb=stb):
                h = (n // 2) // SEG * SEG
                vis2 = stb["vis2"]
                nc.vector.tensor_tensor_scan(vis2[:, h:n], stb["ta"][:, h:n],
                                             bnd[:, h:n], vis2[:, h - 1:h],
                                             OP.mult, OP.max)

            def g5(j=j, n=n, sl=sl, stb=stb):
                o2 = ob_pool.tile([P, n], BF, tag="ob")
                nc.vector.tensor_mul(o2[:], stb["vis2"][:], stb["nv1"][:])
                nc.sync.dma_start(outB_aps[1][:, sl], o2[:])
                nv2 = nvb_pool.tile([P, n], BF, tag="nvb")
                nc.vector.tensor_scalar(nv2[:], stb["vis2"][:], -1.0, 1.0,
                                        OP.mult, OP.add)
                stb["nv2"] = nv2

            def g6(j=j, n=n, sl=sl, stb=stb):
                a3 = sb_pool.tile([P, n], BF, tag="sb")
                nc.vector.tensor_mul(a3[:], stb["a2"][:], stb["nv2"][:])
                ta = ta_pool.tile([P, B_TILE + 1], BF, tag="ta")
                nc.vector.tensor_scalar(ta[:, 1:n + 1], a3[:], -1.0, 1.0,
                                        OP.mult, OP.add)
                stb["ta"] = ta

            def g7a(j=j, n=n, sl=sl, stb=stb):
                h = (n // 2) // SEG * SEG
                vis3 = visb_pool.tile([P, n], BF, tag="visb")
                nc.vector.tensor_tensor_scan(vis3[:, 0:h], stb["ta"][:, 0:h],
                                             bnd[:, 0:h], 1.0, OP.mult, OP.max)
                stb["vis3"] = vis3

            def g7(j=j, n=n, sl=sl, stb=stb):
                h = (n // 2) // SEG * SEG
                vis3 = stb["vis3"]
                nc.vector.tensor_tensor_scan(vis3[:, h:n], stb["ta"][:, h:n],
                                             bnd[:, h:n], vis3[:, h - 1:h],
                                             OP.mult, OP.max)

            def g8(j=j, n=n, sl=sl, stb=stb):
                o3 = ob_pool.tile([P, n], BF, tag="ob")
                nc.vector.tensor_mul(o3[:], stb["vis3"][:], stb["nv2"][:])
                nc.sync.dma_start(outB_aps[2][:, sl], o3[:])

            yield from (g1, g2a, g2, g3, g4a, g4, g5, g6, g7a, g7, g8)

    bq = b_groups()
    n_groups = 11 * len(B_SIZES)
    n_slots = 6 * (len(A_SIZES) + 2)
    popped = [0, 0]  # groups popped, slots seen

    DELAY_SLOTS = 8
    eff_slots = n_slots - DELAY_SLOTS

    def pop_b():
        popped[1] += 1
        s = popped[1] - DELAY_SLOTS
        want = 0 if s <= 0 else (s * n_groups + eff_slots - 1) // eff_slots
        while popped[0] < want:
            try:
                next(bq)()
            except StopIteration:
                return
            popped[0] += 1

    # ---------------- schedule -----------------------------------------
    NT = len(A_SIZES)
    for r in range(NT + 2):
        if r < NT:
            st_a_ln(r, after_dma=load_consts if r == 0 else None)
        pop_b()
        if 1 <= r <= NT:
            st_b_pre(r - 1)
        pop_b()
        if r < NT:
            st_a_exp(r)
        pop_b()
        if 2 <= r:
            st_c_pre(r - 2)
        pop_b()
        if 1 <= r <= NT:
            st_b_exp(r - 1)
        pop_b()
        if 2 <= r:
            st_c_exp(r - 2)
        pop_b()
    for _ in range(11 * len(B_SIZES)):
        pop_b()


def _build():
    nc = bacc.Bacc("TRN2", target_bir_lowering=False, debug=False,
                   num_devices=N_CORES)
    inA_ap = nc.dram_tensor("alphaA", [P, N_A], BF, kind="ExternalInput").ap()
    inB_ap = nc.dram_tensor("alphaB", [P, N_B], BF, kind="ExternalInput").ap()
    tri_ap = nc.dram_tensor("tri", [P, P], BF, kind="ExternalInput").ap()
    bnd_ap = nc.dram_tensor("bnd", [P, max(B_SIZES)], BF,
                            kind="ExternalInput").ap()
    outA_aps = [
        nc.dram_tensor(f"outA{c}", [P, N_A], BF, kind="ExternalOutput").ap()
        for c in range(3)
    ]
    outB_aps = [
        nc.dram_tensor(f"outB{c}", [P, N_B], BF, kind="ExternalOutput").ap()
        for c in range(3)
    ]
    with tile.TileContext(nc) as tc:
        _alpha_kernel(tc, outA_aps, outB_aps, inA_ap, inB_ap, tri_ap, bnd_ap)
    nc.compile()
    return nc


def _get_nc():
    if "nc" not in _COMPILED:
        _COMPILED["nc"] = _build()
    return _COMPILED["nc"]


def _run(alpha_imgs: np.ndarray, trace: bool = False):
    nc = _get_nc()
    tri = _tri_matrix().astype(BF16)
    bndv = np.zeros((P, max(B_SIZES)), dtype=np.float32)
    bndv[:, 0::SEG] = 1.0
    bndv = bndv.astype(BF16)
    # clamp below 1: bf16 rounding can hit 1.0 exactly, and the A-path's
    # Ln(1-a) -> -inf would poison the matmul (0 * -inf = NaN)
    a = np.minimum(np.asarray(alpha_imgs)[:, :, 0], np.float32(1 - 2**-8))
    a = a.astype(BF16)  # [B, D, H, W] bf16
    in_maps = []
    for c in range(N_CORES):
        h0 = c * H_SH
        shA = np.ascontiguousarray(a[:, :, h0:h0 + R_A, :]).reshape(P, N_A)
        # B rows -> pixel-major: [B, D, R_B, W] -> [B, R_B, W, D] -> [128, -1]
        shB = np.ascontiguousarray(
            a[:, :, h0 + R_A:h0 + H_SH, :].transpose(0, 2, 3, 1)
        ).reshape(P, N_B)
        in_maps.append({"alphaA": shA, "alphaB": shB, "tri": tri, "bnd": bndv})
    res = None
    backoffs = [3.0, 10.0, 30.0, 60.0]
    for attempt in range(len(backoffs) + 1):
        try:
            res = run_bass_kernel_spmd(
                nc, in_maps, core_ids=list(range(N_CORES)), trace=trace
            )
            break
        except Exception:
            if attempt == len(backoffs):
                raise
            import time

            time.sleep(backoffs[attempt])
    out = np.empty((B, D, 3, H, W), dtype=np.float32)
    for c in range(N_CORES):
        r = res.results[c]
        h0 = c * H_SH
        oA = np.stack([r["outA0"], r["outA1"], r["outA2"]], axis=1)
        oA = oA.astype(np.float32).reshape(B, D, 3, R_A, W)
        out[:, :, :, h0:h0 + R_A, :] = oA
        # [128, 3, N_B] pixel-major -> [B, D, 3, R_B, W]
        oB = np.stack([r["outB0"], r["outB1"], r["outB2"]], axis=1)
        oB = oB.astype(np.float32)
        ob = oB.reshape(P, 3, N_B // SEG, SEG)           # [part, l, px/part, d]
        ob = ob.transpose(1, 0, 2, 3).reshape(3, B, R_B, W, SEG)
        out[:, :, :, h0 + R_A:h0 + H_SH, :] = ob.transpose(1, 4, 0, 2, 3)
    return out, res


def kernel(alpha_imgs: np.ndarray) -> np.ndarray:
    out, _ = _run(alpha_imgs, trace=False)
    return out

